# revision 1
# baseline (speedup 1.0000x reference)
"""Trainium2 Bass kernel for the custom LSTM problem.

Strategy: tensor-parallel over the 4H gate dimension across 8 NeuronCores.
Each core j owns H-coords [128j, 128j+128) of all four gates (layout
[i|f|o|g], 128 each = 512 gate columns). Per step each core computes its
512-column slice of z = xp_t + h @ Wh.T for the full batch (B=128), updates
its c/h chunk, and the per-step AllGather reassembles the full hidden state
h^T for the next step's matmuls. The input projection is folded into a
projected embedding table (emb @ Wi_j.T + b_j, [V, 512] per core) built
on-device once; per-step xp tiles are fetched from it with dma_gather.
"""

import os
import time
import numpy as np
import ml_dtypes

import concourse.bass as bass
import concourse.mybir as mybir
import concourse.tile as tile
from concourse import bacc
from concourse.bass_utils import run_bass_kernel_spmd
from concourse.masks import make_identity

V, E, H, B, T_FULL, O = 32000, 512, 1024, 128, 512, 1
VPAD = 32768
NCORES = 8
GS = 512   # per-core gate-slice width (4 gates x 128)
HC = 128   # per-core hidden chunk
PAD_IDX = 0

f32 = mybir.dt.float32
bf16 = mybir.dt.bfloat16
i16 = mybir.dt.int16

LAST_EXEC_NS = None

_built = {}


def _build(t_steps, debug=False):
    key = (t_steps, debug)
    if key in _built:
        return _built[key]
    assert t_steps % 4 == 0
    nblk = t_steps // 4
    nidx_cols = (B * t_steps) // 16

    nc = bacc.Bacc("TRN2", target_bir_lowering=False, debug=False,
                   num_devices=NCORES)
    dbg = {}
    if debug:
        dbg["z0"] = nc.dram_tensor("d_z0", [B, GS], f32, kind="ExternalOutput")
        dbg["s0"] = nc.dram_tensor("d_s0", [B, 384], f32, kind="ExternalOutput")
        dbg["c0"] = nc.dram_tensor("d_c0", [B, HC], f32, kind="ExternalOutput")
        dbg["h0"] = nc.dram_tensor("d_h0", [B, HC], f32, kind="ExternalOutput")
        dbg["hT1"] = nc.dram_tensor("d_hT1", [128, H // 128 * HC], bf16,
                                    kind="ExternalOutput")
        dbg["z1"] = nc.dram_tensor("d_z1", [B, GS], f32, kind="ExternalOutput")
        dbg["oacc"] = nc.dram_tensor("d_oacc", [B, HC], f32,
                                     kind="ExternalOutput")

    embV_d = nc.dram_tensor("embV", [VPAD, E], bf16, kind="ExternalInput")
    wiT_d = nc.dram_tensor("wiT", [E, GS], bf16, kind="ExternalInput")
    whT_d = nc.dram_tensor("whT", [H, GS], bf16, kind="ExternalInput")
    bias_d = nc.dram_tensor("biasj", [B, GS], f32, kind="ExternalInput")
    idx_d = nc.dram_tensor("idx16", [128, nidx_cols], i16, kind="ExternalInput")
    mask_d = nc.dram_tensor("maskv", [B, t_steps], f32, kind="ExternalInput")
    fcw_d = nc.dram_tensor("fcw", [HC, 1], f32, kind="ExternalInput")
    fcb_d = nc.dram_tensor("fcb", [1, B], f32, kind="ExternalInput")
    y_d = nc.dram_tensor("y", [1, B], f32, kind="ExternalOutput")

    with tile.TileContext(nc) as tc:
        with (
            tc.tile_pool(name="const", bufs=1) as constp,
            tc.tile_pool(name="work", bufs=3) as work,
            tc.tile_pool(name="state", bufs=1) as state,
            tc.tile_pool(name="xps", bufs=3) as xps,
            tc.tile_pool(name="hts", bufs=2) as hts,
            tc.tile_pool(name="zpsum", bufs=4, space="PSUM") as zpsum,
            tc.tile_pool(name="tpsum", bufs=2, space="PSUM") as tpsum,
            tc.tile_pool(name="agin", bufs=3, space="DRAM") as agin,
            tc.tile_pool(name="agout", bufs=3, space="DRAM") as agout,
        ):
            # ---- constants into SBUF ----
            wiT_sb = constp.tile([128, E // 128, GS], bf16, name="wiT_sb")
            nc.sync.dma_start(
                wiT_sb[:], wiT_d.ap().rearrange("(ko p) n -> p ko n", p=128))
            whT_sb = constp.tile([128, H // 128, GS], bf16, name="whT_sb")
            nc.sync.dma_start(
                whT_sb[:], whT_d.ap().rearrange("(ko p) n -> p ko n", p=128))
            bias_sb = constp.tile([B, GS], f32, name="bias_sb")
            nc.sync.dma_start(bias_sb[:], bias_d.ap())
            mask_sb = constp.tile([B, t_steps], f32, name="mask_sb")
            nc.sync.dma_start(mask_sb[:], mask_d.ap())
            idx_sb = constp.tile([128, nidx_cols], i16, name="idx_sb")
            nc.sync.dma_start(idx_sb[:], idx_d.ap())
            fcw_sb = constp.tile([HC, 1], f32, name="fcw_sb")
            nc.sync.dma_start(fcw_sb[:], fcw_d.ap())
            fcb_sb = constp.tile([1, B], f32, name="fcb_sb")
            nc.sync.dma_start(fcb_sb[:], fcb_d.ap())
            ident = constp.tile([128, 128], f32, name="ident")
            make_identity(nc, ident[:])
            # bias as a K=1 matmul operand pair: ones[1,B].T @ brow[1,GS]
            # accumulates b_j into the z PSUM, so ACT reads PSUM directly
            # (drops the DVE z-add from the serial chain)
            ones_sb = constp.tile([1, B], bf16, name="ones_sb")
            nc.vector.memset(ones_sb[:], 1.0)
            brow_sb = constp.tile([1, GS], bf16, name="brow_sb")
            nc.vector.tensor_copy(brow_sb[:], bias_sb[0:1, :])

            # ---- recurrence (input projection fused into each step) ----
            c_t = state.tile([B, HC], f32, name="c_t")
            nc.vector.memset(c_t[:], 0.0)
            oacc = state.tile([B, HC], f32, name="oacc")
            nc.vector.memset(oacc[:], 0.0)
            hT_all = hts.tile([128, H // 128, HC], bf16, tag="hTall",
                              name="hTall_init")
            nc.vector.memset(hT_all[:], 0.0)

            xe_tiles = {}

            def issue_gather(blk):
                if blk >= nblk:
                    return
                xe = xps.tile([128, E // 128, 512], bf16, tag="xe",
                              name=f"xe{blk}")
                nc.gpsimd.dma_gather(
                    out_ap=xe[:],
                    in_ap=embV_d.ap(),
                    idxs_ap=idx_sb[:, 32 * blk:32 * (blk + 1)],
                    num_idxs=512,
                    num_idxs_reg=512,
                    elem_size=E,
                    transpose=True,
                )
                xe_tiles[blk] = xe

            issue_gather(0)
            issue_gather(1)

            for t in range(t_steps):
                blk, off = divmod(t, 4)
                if off == 0:
                    issue_gather(blk + 2)
                xe = xe_tiles[blk]

                ps = zpsum.tile([B, GS], f32, tag="zps", name=f"zps{t}")
                # input-projection + bias: no dependency on h -> runs in the
                # AllGather wait window
                for ke in range(E // 128):
                    nc.tensor.matmul(
                        ps[:], xe[:, ke, 128 * off:128 * (off + 1)],
                        wiT_sb[:, ke, :],
                        start=(ke == 0), stop=False)
                nc.tensor.matmul(ps[:], ones_sb[:], brow_sb[:],
                                 start=False, stop=False)
                for k in range(H // 128):
                    nc.tensor.matmul(ps[:], hT_all[:, k, :], whT_sb[:, k, :],
                                     start=False, stop=(k == H // 128 - 1))
                if debug and t == 0:
                    zd = work.tile([B, GS], f32, tag="zd", name=f"zd{t}")
                    nc.vector.tensor_copy(zd[:], ps[:])
                    nc.sync.dma_start(dbg["z0"].ap(), zd[:])
                if debug and t == 1:
                    zd = work.tile([B, GS], f32, tag="zd", name=f"zd{t}")
                    nc.vector.tensor_copy(zd[:], ps[:])
                    nc.sync.dma_start(dbg["z1"].ap(), zd[:])

                th = work.tile([B, 384], f32, tag="th", name=f"th{t}")
                nc.scalar.activation(th[:], ps[:, 0:384],
                                     mybir.ActivationFunctionType.Tanh,
                                     scale=0.5)
                s = work.tile([B, 384], f32, tag="s", name=f"s{t}")
                nc.vector.tensor_scalar(s[:], th[:], 0.5, 0.5,
                                        mybir.AluOpType.mult,
                                        mybir.AluOpType.add)
                g = work.tile([B, HC], f32, tag="g", name=f"g{t}")
                nc.scalar.activation(g[:], ps[:, 384:512],
                                     mybir.ActivationFunctionType.Tanh)

                ig = work.tile([B, HC], f32, tag="ig", name=f"ig{t}")
                nc.vector.tensor_mul(ig[:], s[:, 0:128], g[:])
                cf = work.tile([B, HC], f32, tag="cf", name=f"cf{t}")
                nc.vector.tensor_mul(cf[:], c_t[:], s[:, 128:256])
                nc.vector.tensor_add(c_t[:], cf[:], ig[:])
                thc = work.tile([B, HC], f32, tag="thc", name=f"thc{t}")
                nc.scalar.activation(thc[:], c_t[:],
                                     mybir.ActivationFunctionType.Tanh)
                h = work.tile([B, HC], f32, tag="h", name=f"h{t}")
                nc.vector.tensor_mul(h[:], s[:, 256:384], thc[:])
                if debug and t == 0:
                    nc.sync.dma_start(dbg["s0"].ap(), s[:])
                    nc.sync.dma_start(dbg["c0"].ap(), c_t[:])
                    nc.sync.dma_start(dbg["h0"].ap(), h[:])

                nc.vector.scalar_tensor_tensor(
                    oacc[:], h[:], mask_sb[:, t:t + 1], oacc[:],
                    mybir.AluOpType.mult, mybir.AluOpType.add)

                if t < t_steps - 1:
                    tp = tpsum.tile([HC, B], f32, tag="tp", name=f"tp{t}")
                    nc.tensor.transpose(tp[:], h[:], ident[:])
                    hTj = work.tile([HC, B], bf16, tag="hTj", name=f"hTj{t}")
                    nc.vector.tensor_copy(hTj[:], tp[:])
                    ib = agin.tile([HC, B], bf16, tag="ib", name=f"ib{t}")
                    nc.sync.dma_start(ib[:], hTj[:])
                    ob = agout.tile([128 * NCORES, B], bf16, tag="ob",
                                    name=f"ob{t}")
                    nc.gpsimd.collective_compute(
                        "AllGather",
                        mybir.AluOpType.bypass,
                        replica_groups=[list(range(NCORES))],
                        ins=[ib.opt()],
                        outs=[ob.opt()],
                    )
                    hT_all = hts.tile([128, H // 128, HC], bf16, tag="hTall",
                                      name=f"hTall{t}")
                    obr = ob.opt().rearrange("(k p) b -> p k b", p=128)
                    nc.sync.dma_start(hT_all[:, 0:4, :], obr[:, 0:4, :])
                    nc.sync.dma_start(hT_all[:, 4:8, :], obr[:, 4:8, :])
                    if debug and t == 0:
                        nc.sync.dma_start(
                            dbg["hT1"].ap().rearrange("p (k b) -> p k b", k=8),
                            hT_all[:])

            # ---- phase 3: masked output -> fc partial -> AllReduce ----
            if debug:
                nc.sync.dma_start(dbg["oacc"].ap(), oacc[:])
            tpo = tpsum.tile([HC, B], f32, tag="tp", name="tpo")
            nc.tensor.transpose(tpo[:], oacc[:], ident[:])
            oT = work.tile([HC, B], f32, tag="oT", name="oT")
            nc.vector.tensor_copy(oT[:], tpo[:])
            fps = tpsum.tile([1, B], f32, tag="fps", name="fps")
            nc.tensor.matmul(fps[:], fcw_sb[:], oT[:], start=True, stop=True)
            fsb = work.tile([1, B], f32, tag="fsb", name="fsb")
            nc.vector.tensor_copy(fsb[:], fps[:])
            arin = agin.tile([1, B], f32, tag="arin", name="arin")
            nc.sync.dma_start(arin[:], fsb[:])
            arout = agout.tile([1, B], f32, tag="arout",
                               name="arout")
            nc.gpsimd.collective_compute(
                "AllReduce",
                mybir.AluOpType.add,
                replica_groups=[list(range(NCORES))],
                ins=[arin.opt()],
                outs=[arout.opt()],
            )
            ssum = work.tile([1, B], f32, tag="ssum", name="ssum")
            nc.sync.dma_start(ssum[:], arout.opt())
            ysb = work.tile([1, B], f32, tag="ysb", name="ysb")
            nc.vector.tensor_add(ysb[:], ssum[:], fcb_sb[:])
            nc.sync.dma_start(y_d.ap(), ysb[:])

    nc.compile()
    _built[key] = nc
    return nc


def _prep_inputs(x, lengths, emb, W_ii, W_hi, b_i, W_if, W_hf, b_f,
                 W_ig, W_hg, b_g, W_io, W_ho, b_o, fc_w, fc_b, t_steps):
    """Host-side layout prep; returns per-core in_maps."""
    x = np.asarray(x).astype(np.int64)[:, :t_steps]
    lengths = np.asarray(lengths).astype(np.int64)
    emb = np.asarray(emb, dtype=np.float32).copy()
    emb[PAD_IDX] = 0.0

    embV = np.zeros((VPAD, E), dtype=ml_dtypes.bfloat16)
    embV[:V] = emb.astype(ml_dtypes.bfloat16)

    # t-major token order: global idx g = t*B + b
    xt = np.ascontiguousarray(x.T)            # [t, B]
    flat = xt.reshape(-1).astype(np.int16)     # ids < 32000 fit in int16
    nidx_cols = (B * t_steps) // 16
    # [16, cols] block replicated down all 128 partitions — the SWDGE Q7
    # cores read the block through queue-dependent 16-partition windows.
    idx16 = np.tile(flat.reshape(nidx_cols, 16).T, (8, 1))

    maskv = (lengths[:, None] == (np.arange(t_steps)[None, :] + 1)).astype(
        np.float32)

    fc_w = np.asarray(fc_w, dtype=np.float32).reshape(O, H)
    fc_b = np.asarray(fc_b, dtype=np.float32).reshape(O)
    fcb_rep = np.full((1, B), fc_b[0], dtype=np.float32)

    in_maps = []
    for j in range(NCORES):
        hj = slice(128 * j, 128 * (j + 1))
        Wi_j = np.concatenate(
            [np.asarray(W_ii)[hj], np.asarray(W_if)[hj],
             np.asarray(W_io)[hj], np.asarray(W_ig)[hj]], axis=0)
        Wh_j = np.concatenate(
            [np.asarray(W_hi)[hj], np.asarray(W_hf)[hj],
             np.asarray(W_ho)[hj], np.asarray(W_hg)[hj]], axis=0)
        b_j = np.concatenate(
            [np.asarray(b_i)[hj], np.asarray(b_f)[hj],
             np.asarray(b_o)[hj], np.asarray(b_g)[hj]], axis=0)
        in_maps.append({
            "embV": embV,
            "wiT": np.ascontiguousarray(Wi_j.T).astype(ml_dtypes.bfloat16),
            "whT": np.ascontiguousarray(Wh_j.T).astype(ml_dtypes.bfloat16),
            "biasj": np.tile(b_j.astype(np.float32)[None, :], (B, 1)),
            "idx16": idx16,
            "maskv": maskv,
            "fcw": np.ascontiguousarray(
                fc_w[0, hj].astype(np.float32).reshape(HC, 1)),
            "fcb": fcb_rep,
        })
    return in_maps


def kernel(**inputs):
    global LAST_EXEC_NS
    t_steps = int(os.environ.get("KERNEL_T", T_FULL))
    nc = _build(t_steps)
    in_maps = _prep_inputs(t_steps=t_steps, **inputs)
    trace = bool(int(os.environ.get("KERNEL_TRACE", "0")))
    res = run_bass_kernel_spmd(nc, in_maps, core_ids=list(range(NCORES)),
                               trace=trace)
    LAST_EXEC_NS = res.exec_time_ns
    y = np.asarray(res.results[0]["y"], dtype=np.float32).reshape(B)
    return y.reshape(B, O)

